# revision 1
# baseline (speedup 1.0000x reference)
"""Trainium2 Bass kernel for nn_DenseStationaryQMatrixDecoder.

Reference math: Q = rownorm(exp(logQ) * (1-I)) - I  (a 4x4 CTMC rate matrix),
output = broadcast(row0(expm(Q*1000)), (V, S, A)).  expm(Q*1000) converges to
the rank-1 stationary matrix 1*pi^T, so every output element is pi[a].

Device strategy (per core, 8 cores data-parallel over V):
  1. Compute R = 0.5*(P + I) on-chip from logQ (P = the normalized hop
     matrix); R is a strictly-positive stochastic matrix whose stationary
     distribution equals pi.
  2. Converge via repeated squaring: R^(2^NSQ) -> rows == pi.  Squaring
     without explicit transposes: keep (X, X^T); X2 = matmul(lhsT=X^T, rhs=X),
     X2^T = matmul(lhsT=X, rhs=X^T).
  3. Broadcast pi across 128 partitions with a ones-vector matmul, tile it
     along the free dim into a [128, FREE] SBUF pattern tile.
  4. DMA the same pattern tile to all CHUNKS contiguous DRAM regions of this
     core's output shard (the bytes are identical everywhere).
"""

import sys

if "/opt/trn_rl_repo" not in sys.path:
    sys.path.insert(0, "/opt/trn_rl_repo")

import numpy as np

A = 4
V = 512
S = 8192
N_CORES = 8
PER_CORE = V * S * A // N_CORES  # 2,097,152 f32 = 8 MiB
P128 = 128
FREE = 4096                      # pattern tile free size (f32)
REP = FREE // A
CHUNKS = PER_CORE // (P128 * FREE)
NSQ = 12                         # R^(2^12); |lam2|^4096 == 0 for any sane input

_cache = {}


def _build():
    import concourse.bacc as bacc
    import concourse.mybir as mybir
    import concourse.tile as tile

    f32 = mybir.dt.float32
    AF = mybir.ActivationFunctionType
    AX = mybir.AxisListType
    OP = mybir.AluOpType

    nc = bacc.Bacc(
        "TRN2", target_bir_lowering=False, debug=False, num_devices=N_CORES
    )
    logq = nc.dram_tensor("logq", [A, A], f32, kind="ExternalInput").ap()
    consts = nc.dram_tensor("consts", [A, 2 * A], f32, kind="ExternalInput").ap()
    ones_r = nc.dram_tensor("ones_r", [1, P128], f32, kind="ExternalInput").ap()
    out = nc.dram_tensor(
        "out", [CHUNKS, P128, FREE], f32, kind="ExternalOutput"
    ).ap()

    with tile.TileContext(nc) as tc:
        with (
            tc.tile_pool(name="small", bufs=1) as sp,
            tc.tile_pool(name="loop", bufs=2) as lp,
            tc.tile_pool(name="patt", bufs=1) as pp,
            tc.tile_pool(name="ps1", bufs=1, space="PSUM") as ps1,
            tc.tile_pool(name="ps2", bufs=2, space="PSUM") as ps2,
        ):
            lq = sp.tile([A, A], f32)
            nc.sync.dma_start(out=lq[:], in_=logq)
            cst = sp.tile([A, 2 * A], f32)
            nc.sync.dma_start(out=cst[:], in_=consts)
            ones = sp.tile([1, P128], f32)
            nc.sync.dma_start(out=ones[:], in_=ones_r)
            offmask = cst[:, 0:A]     # 1 - I
            halfeye = cst[:, A : 2 * A]  # 0.5 * I

            E = sp.tile([A, A], f32)
            nc.scalar.activation(out=E[:], in_=lq[:], func=AF.Exp)
            Eo = sp.tile([A, A], f32)
            nc.vector.tensor_mul(out=Eo[:], in0=E[:], in1=offmask)
            s = sp.tile([A, 1], f32)
            nc.vector.reduce_sum(out=s[:], in_=Eo[:], axis=AX.X)
            r = sp.tile([A, 1], f32)
            nc.vector.reciprocal(out=r[:], in_=s[:])

            # X = R = 0.5*P + 0.5*I  where P = diag(r) @ Eo
            xh = sp.tile([A, A], f32)
            nc.vector.tensor_scalar(
                out=xh[:], in0=Eo[:], scalar1=r[:], scalar2=0.5,
                op0=OP.mult, op1=OP.mult,
            )
            X0 = sp.tile([A, A], f32)
            nc.vector.tensor_add(out=X0[:], in0=xh[:], in1=halfeye)

            # X^T = R^T = Eo^T @ diag(0.5*r) + 0.5*I   (no PE transpose needed)
            dgr = sp.tile([A, A], f32)
            nc.vector.tensor_scalar_mul(out=dgr[:], in0=halfeye, scalar1=r[:])
            pt = ps1.tile([A, A], f32)
            nc.tensor.matmul(pt[:], lhsT=Eo[:], rhs=dgr[:], start=True, stop=True)
            XT0 = sp.tile([A, A], f32)
            nc.vector.tensor_add(out=XT0[:], in0=pt[:], in1=halfeye)

            Xa, XTa = X0, XT0
            for _ in range(NSQ):
                pa = ps2.tile([A, A], f32)
                pb = ps2.tile([A, A], f32)
                nc.tensor.matmul(pa[:], lhsT=XTa[:], rhs=Xa[:], start=True, stop=True)
                nc.tensor.matmul(pb[:], lhsT=Xa[:], rhs=XTa[:], start=True, stop=True)
                Xn = lp.tile([A, A], f32)
                XTn = lp.tile([A, A], f32)
                nc.scalar.copy(out=Xn[:], in_=pa[:])
                nc.vector.tensor_copy(out=XTn[:], in_=pb[:])
                Xa, XTa = Xn, XTn

            # pi (row 0 of converged X) broadcast to 128 partitions:
            # ones(128,1) @ row0(1,4)
            pbig = ps1.tile([P128, A], f32)
            nc.tensor.matmul(
                pbig[:], lhsT=ones[:], rhs=Xa[0:1, :], start=True, stop=True
            )
            seed = sp.tile([P128, A], f32)
            nc.vector.tensor_copy(out=seed[:], in_=pbig[:])

            patt = pp.tile([P128, FREE], f32)
            patt3 = patt[:].rearrange("p (r a) -> p r a", a=A)
            seed3 = seed[:].unsqueeze(1).to_broadcast((P128, REP, A))
            nc.vector.tensor_copy(out=patt3, in_=seed3)

            for c in range(CHUNKS):
                nc.sync.dma_start(out=out[c], in_=patt[:])

    nc.compile()
    return nc


def _get_nc():
    if "nc" not in _cache:
        _cache["nc"] = _build()
    return _cache["nc"]


def _in_map(log_Q_matrix_AxA):
    logq = np.ascontiguousarray(
        np.asarray(log_Q_matrix_AxA, dtype=np.float32).reshape(A, A)
    )
    eye = np.eye(A, dtype=np.float32)
    consts = np.ascontiguousarray(
        np.concatenate([1.0 - eye, 0.5 * eye], axis=1)
    )
    ones = np.ones((1, P128), dtype=np.float32)
    return {"logq": logq, "consts": consts, "ones_r": ones}


def kernel(
    embeddings_VxD=None, site_positions_SxC=None, log_Q_matrix_AxA=None, **_unused
):
    from concourse.bass_utils import run_bass_kernel_spmd

    nc = _get_nc()
    im = _in_map(log_Q_matrix_AxA)
    res = run_bass_kernel_spmd(
        nc, [dict(im) for _ in range(N_CORES)], core_ids=list(range(N_CORES))
    )
    parts = [r["out"].reshape(V // N_CORES, S, A) for r in res.results]
    return np.concatenate(parts, axis=0)
